# revision 1
# baseline (speedup 1.0000x reference)
"""Trainium2 Bass kernel for nn_DMGAGRUcell (GRU cell with graph-conv gates).

Math (per batch b):
  x    = [inputs | hx]                      (N, 66)
  x1   = S @ x, x2 = adp[b] @ x             (diffusion + adaptive hop)
  ru   = sigmoid([x|x1|x2]_interleaved @ W_ru);  r, u = split(ru)
  c    = tanh([x|x1|x2']_interleaved @ W_c)  with x' = [inputs | r*hx]
  out  = u*hx + (1-u)*c

Sharding: 2 batches per core x 8 cores (data parallel over B=16).
Device layout is feature-major (transposed): all gconv outputs are computed
as x1T = x.T @ S.T etc. with the small x as the PE stationary operand and the
big matrix streaming; adp[b] (bf16, host-pre-transposed) stays resident in
SBUF so HBM reads it once per batch. The dominant-magnitude gate chunks
(x0 @ W0, rh @ W) run in fp32; the small x1/x2 chunks run in bf16.
"""

import os
import numpy as np
import ml_dtypes

BF16 = ml_dtypes.bfloat16

N = 2048
B = 16
D_IN = 2
UNITS = 64
F = 66
B_LOC = 2          # batches per core
N_CORES = 8
KC = 16            # k chunks of 128 nodes
NS = 4             # 512-wide output slabs

_CACHE = {}


def _build():
    if "nc" in _CACHE:
        return _CACHE["nc"]

    from contextlib import ExitStack
    import concourse.mybir as mybir
    import concourse.tile as tile
    from concourse import bacc

    f32 = mybir.dt.float32
    bf = mybir.dt.bfloat16
    f8 = mybir.dt.float8e4
    AF = mybir.ActivationFunctionType

    nc = bacc.Bacc("TRN2", target_bir_lowering=False, debug=False,
                   num_devices=N_CORES)

    adpT_d = nc.dram_tensor("adpT", [B_LOC, KC, 128, N], f8, kind="ExternalInput")
    sT_d = nc.dram_tensor("sT", [KC, 128, N], bf, kind="ExternalInput")
    xnm_d = nc.dram_tensor("xnm", [B_LOC, 128, KC, F], bf, kind="ExternalInput")
    x0Tb_d = nc.dram_tensor("x0Tb", [B_LOC, F, N], bf, kind="ExternalInput")
    hxTf_d = nc.dram_tensor("hxTf", [B_LOC, UNITS, N], f32, kind="ExternalInput")
    wru0_d = nc.dram_tensor("wru0b", [F, 128], bf, kind="ExternalInput")
    wru1_d = nc.dram_tensor("wru1b", [F, 128], bf, kind="ExternalInput")
    wru2_d = nc.dram_tensor("wru2b", [F, 128], bf, kind="ExternalInput")
    wcinp_d = nc.dram_tensor("wcinpb", [D_IN, UNITS], bf, kind="ExternalInput")
    wcrh_d = nc.dram_tensor("wcrhb", [UNITS, UNITS], bf, kind="ExternalInput")
    wc1_d = nc.dram_tensor("wc1b", [F, UNITS], bf, kind="ExternalInput")
    wc2_d = nc.dram_tensor("wc2b", [F, UNITS], bf, kind="ExternalInput")
    id_d = nc.dram_tensor("ident", [UNITS, UNITS], bf, kind="ExternalInput")
    out_d = nc.dram_tensor("outT", [B_LOC, UNITS, N], f32, kind="ExternalOutput")

    with tile.TileContext(nc) as tc, ExitStack() as ctx:
        spool = ctx.enter_context(tc.tile_pool(name="spool", bufs=1))
        apool = ctx.enter_context(tc.tile_pool(name="apool", bufs=16))
        cpool = ctx.enter_context(tc.tile_pool(name="cpool", bufs=1))
        wpool = ctx.enter_context(tc.tile_pool(name="wpool", bufs=1))
        w2pool = ctx.enter_context(tc.tile_pool(name="w2pool", bufs=2))
        pp = ctx.enter_context(tc.tile_pool(name="pp", bufs=8, space="PSUM"))

        # DMA order tracks first use: xnm0, s0 (the first matmuls' inputs),
        # xnm1, the rest of the S stream, then the late-needed inputs
        binp = {}
        xnm0 = w2pool.tile([128, KC, F], bf, tag="xnm", name="xnm0")
        nc.sync.dma_start(xnm0[:], xnm_d[0])
        s_tiles = [spool.tile([128, N], bf, tag="s0", name="s0")]
        for q in range(NS):
            qsl = slice(q * 512, (q + 1) * 512)
            nc.sync.dma_start(s_tiles[0][:, qsl], sT_d[0][:, qsl])
        xnm1 = w2pool.tile([128, KC, F], bf, tag="xnm", name="xnm1")
        nc.sync.dma_start(xnm1[:], xnm_d[1])
        for k in range(1, KC):
            t = spool.tile([128, N], bf, tag=f"s{k}", name=f"s{k}")
            nc.sync.dma_start(t[:], sT_d[k])
            s_tiles.append(t)
        binp[0] = [xnm0]
        binp[1] = [xnm1]
        for b in range(B_LOC):
            x0Tb = w2pool.tile([F, N], bf, tag="x0Tb", name=f"x0Tb{b}")
            nc.sync.dma_start(x0Tb[:], x0Tb_d[b])
            hxTf = wpool.tile([UNITS, N], f32, tag="hxTf", name=f"hxTf{b}")
            nc.sync.dma_start(hxTf[:], hxTf_d[b])
            binp[b] += [x0Tb, hxTf]

        def const(name, dram, shape, dt):
            t = cpool.tile(shape, dt, tag=name, name=name)
            nc.sync.dma_start(t[:], dram[:])
            return t

        wru0 = const("wru0", wru0_d, [F, 128], bf)
        wru1 = const("wru1", wru1_d, [F, 128], bf)
        wru2 = const("wru2", wru2_d, [F, 128], bf)
        wcinp = const("wcinp", wcinp_d, [D_IN, UNITS], bf)
        wcrh = const("wcrh", wcrh_d, [UNITS, UNITS], bf)
        wc1 = const("wc1", wc1_d, [F, UNITS], bf)
        wc2 = const("wc2", wc2_d, [F, UNITS], bf)
        ident = const("ident", id_d, [UNITS, UNITS], bf)

        # warm the ACT function table off the critical path (a function-set
        # switch mid-kernel costs ~1.3us)
        dum = cpool.tile([1, 2], f32, tag="dum", name="dum")
        nc.scalar.activation(dum[0:1, 0:1], ident[0:1, 0:1], AF.Sigmoid)
        nc.scalar.activation(dum[0:1, 1:2], ident[0:1, 0:1], AF.Tanh)

        ADP_SCALE = 1.0 / 2048.0

        def stream_pass(lhs_xnms, rhs_tiles, dsts, pfx, defer_drain=False,
                        scale=None):
            # dsts[i] = lhs_xnms[i].T @ rhs_tiles.T, k-major so several
            # batches' matmuls interleave behind one streamed rhs.
            # Drains stay off the ACT engine: an activation-function switch
            # costs a ~1.3us LoadActFuncSet, so ACT runs only sigmoid/tanh.
            nb = len(lhs_xnms)
            ps = [[pp.tile([F, 512], f32, tag="ps", name=f"ps_{pfx}_{i}_{s}")
                   for s in range(NS)] for i in range(nb)]
            for k in range(KC):
                for i in range(nb):
                    lhsT = lhs_xnms[i][:, k, :]
                    for s in range(NS):
                        nc.tensor.matmul(
                            ps[i][s][:], lhsT,
                            rhs_tiles[k][:, s * 512:(s + 1) * 512],
                            start=(k == 0), stop=(k == KC - 1))
            if defer_drain:
                return ps
            for i in range(nb):
                for s in range(NS):
                    dsl = dsts[i][:, s * 512:(s + 1) * 512]
                    if scale is None:
                        nc.vector.tensor_copy(dsl, ps[i][s][:])
                    else:
                        nc.vector.tensor_scalar_mul(dsl, ps[i][s][:], scale)

        # ---- gconv 1 S-passes, both batches fused behind one S stream ----
        x1Ts = [w2pool.tile([F, N], bf, tag="x1T", name=f"x1T{b}")
                for b in range(B_LOC)]
        stream_pass([binp[0][0], binp[1][0]], s_tiles, x1Ts, "s1")

        for b in range(B_LOC):
            xnm, x0Tb, hxTf = binp[b]

            a_tiles = []
            for k in range(KC):
                t = apool.tile([128, N], f8, tag="adp", name=f"adp_{b}_{k}")
                nc.sync.dma_start(t[:], adpT_d[b, k])
                a_tiles.append(t)

            # ---- gconv 1 adp-pass ----
            x1T = x1Ts[b]
            x2T = w2pool.tile([F, N], bf, tag="x2T")
            stream_pass([xnm], a_tiles, [x2T], f"a1_{b}", scale=ADP_SCALE)

            # ru = sigmoid(x0.W0 + x1.W1 + x2.W2). r and u are computed as
            # separate accumulation groups (W free-dim split) so both land at
            # partitions 0-63 - two-input DVE ops need equal base partitions.
            # ru = sigmoid([x0|x1|x2] @ W_ru): one M=128 accumulation group
            # per slab; r (rows 0-63) and u (rows 64-127) drain via separate
            # sigmoids, u with a shifted partition base down to 0-63.
            # rh = r*hx follows per slab on the DVE; the PE transposes of rh
            # run after all ru matmuls so their input chain is already done.
            ract = wpool.tile([UNITS, N], f32, tag="ract")
            uact = wpool.tile([UNITS, N], f32, tag="uact")
            rhb = wpool.tile([UNITS, N], bf, tag="rhb")
            ru_ps = []
            for s in range(NS):
                sl = slice(s * 512, (s + 1) * 512)
                ps = pp.tile([128, 512], f32, tag="ps", name=f"ps_ru{s}")
                nc.tensor.matmul(ps[:], wru0[:], x0Tb[:, sl], start=True, stop=False)
                nc.tensor.matmul(ps[:], wru1[:], x1T[:, sl], start=False, stop=False)
                nc.tensor.matmul(ps[:], wru2[:], x2T[:, sl], start=False, stop=True)
                nc.scalar.activation(ract[:, sl], ps[0:UNITS, :], AF.Sigmoid)
                nc.vector.tensor_mul(rhb[:, sl], ract[:, sl], hxTf[:, sl])
                ru_ps.append(ps)
            for s in range(NS):
                # u is needed only at the final combine; keep it off the
                # r -> rh -> transpose critical path
                sl = slice(s * 512, (s + 1) * 512)
                nc.scalar.activation(uact[:, sl], ru_ps[s][UNITS:128, :], AF.Sigmoid)
            for k in range(KC):
                pst = pp.tile([128, 1024], bf, tag="ps", name=f"pst_{k}")
                nc.tensor.transpose(
                    pst[:, 0:UNITS], rhb[:, k * 128:(k + 1) * 128], ident[:])
                nc.vector.tensor_copy(xnm[:, k, D_IN:F], pst[:, 0:UNITS])

            # ---- gconv 2 ----
            x1p = w2pool.tile([F, N], bf, tag="x1T")
            x2p = w2pool.tile([F, N], bf, tag="x2T")
            ps1 = stream_pass([xnm], s_tiles, None, f"s2_{b}", defer_drain=True)
            ps2 = stream_pass([xnm], a_tiles, None, f"a2_{b}", defer_drain=True)
            for s in range(NS):
                dsl = slice(s * 512, (s + 1) * 512)
                nc.vector.tensor_copy(x1p[:, dsl], ps1[0][s][:])
                nc.vector.tensor_scalar_mul(x2p[:, dsl], ps2[0][s][:], ADP_SCALE)

            # c = tanh(inp.Wc[0:2] + rh.Wc[2:66] + x1'.Wc1 + x2'.Wc2)
            cT = wpool.tile([UNITS, N], f32, tag="cT")
            outT = wpool.tile([UNITS, N], f32, tag="outT")
            for s in range(NS):
                sl = slice(s * 512, (s + 1) * 512)
                ps = pp.tile([UNITS, 512], f32, tag="ps", name=f"ps_c{s}")
                nc.tensor.matmul(ps[:], wcinp[:], x0Tb[0:D_IN, sl], start=True, stop=False)
                nc.tensor.matmul(ps[:], wcrh[:], rhb[:, sl], start=False, stop=False)
                nc.tensor.matmul(ps[:], wc1[:], x1p[:, sl], start=False, stop=False)
                nc.tensor.matmul(ps[:], wc2[:], x2p[:, sl], start=False, stop=True)
                nc.scalar.activation(cT[:, sl], ps[:], AF.Tanh)
                # out = c + u*(hx - c); alternate slabs between DVE and
                # GpSimd so two dependency chains run in parallel
                eng = nc.vector if s % 2 == 1 else nc.gpsimd
                eng.tensor_sub(outT[:, sl], hxTf[:, sl], cT[:, sl])
                eng.tensor_mul(outT[:, sl], uact[:, sl], outT[:, sl])
                eng.tensor_add(outT[:, sl], outT[:, sl], cT[:, sl])
            nc.sync.dma_start(out_d[b], outT[:])

    nc.compile()
    _CACHE["nc"] = nc
    return nc


def _prep_host(inputs, hx, adp, support_rows, support_cols, support_vals,
               W_ru, W_c):
    xcat = np.concatenate(
        [inputs.reshape(B, N, D_IN), hx.reshape(B, N, UNITS)], axis=2)
    xcat = np.ascontiguousarray(xcat, dtype=np.float32)

    S = np.zeros((N, N), np.float32)
    np.add.at(S, (support_rows, support_cols), support_vals)
    sT = np.ascontiguousarray(S.T).astype(BF16).reshape(KC, 128, N)

    FP8 = ml_dtypes.float8_e4m3fn
    adpT = (np.ascontiguousarray(adp.transpose(0, 2, 1)) * 2048.0).astype(
        FP8).reshape(B, KC, 128, N)

    xnm = xcat.astype(BF16).reshape(B, KC, 128, F).transpose(0, 2, 1, 3)
    xnm = np.ascontiguousarray(xnm)
    x0T = np.ascontiguousarray(xcat.transpose(0, 2, 1))
    x0Tb = x0T.astype(BF16)
    hxTf = np.ascontiguousarray(x0T[:, D_IN:F])

    wru = {
        "wru0b": np.ascontiguousarray(W_ru[0::3]).astype(BF16),
        "wru1b": np.ascontiguousarray(W_ru[1::3]).astype(BF16),
        "wru2b": np.ascontiguousarray(W_ru[2::3]).astype(BF16),
    }
    wc0 = np.ascontiguousarray(W_c[0::3])
    wcd = {
        "wcinpb": np.ascontiguousarray(wc0[0:D_IN]).astype(BF16),
        "wcrhb": np.ascontiguousarray(wc0[D_IN:F]).astype(BF16),
        "wc1b": np.ascontiguousarray(W_c[1::3]).astype(BF16),
        "wc2b": np.ascontiguousarray(W_c[2::3]).astype(BF16),
    }
    ident = np.eye(UNITS, dtype=BF16)

    shared = {"sT": sT, "ident": ident, **wru, **wcd}
    in_maps = []
    for c in range(N_CORES):
        lo, hi = c * B_LOC, (c + 1) * B_LOC
        in_maps.append({
            "adpT": np.ascontiguousarray(adpT[lo:hi]),
            "xnm": np.ascontiguousarray(xnm[lo:hi]),
            "x0Tb": np.ascontiguousarray(x0Tb[lo:hi]),
            "hxTf": np.ascontiguousarray(hxTf[lo:hi]),
            **shared,
        })
    return in_maps


def kernel(inputs, hx, adp, support_rows, support_cols, support_vals,
           W_ru, W_c, time_axis=None):
    from concourse.bass_utils import run_bass_kernel_spmd

    inputs = np.asarray(inputs, dtype=np.float32)
    hx = np.asarray(hx, dtype=np.float32)
    adp = np.asarray(adp, dtype=np.float32)
    support_rows = np.asarray(support_rows)
    support_cols = np.asarray(support_cols)
    support_vals = np.asarray(support_vals, dtype=np.float32)
    W_ru = np.asarray(W_ru, dtype=np.float32)
    W_c = np.asarray(W_c, dtype=np.float32)

    nc = _build()
    in_maps = _prep_host(inputs, hx, adp, support_rows, support_cols,
                         support_vals, W_ru, W_c)

    res = run_bass_kernel_spmd(nc, in_maps, core_ids=list(range(N_CORES)),
                               trace=False)
    _CACHE["last_result"] = res

    out = np.empty((B, N * UNITS), np.float32)
    for c in range(N_CORES):
        outT = res.results[c]["outT"]  # (B_LOC, 64, N)
        for i in range(B_LOC):
            out[c * B_LOC + i] = np.ascontiguousarray(
                outT[i].T).reshape(N * UNITS)
    return out



# revision 4
# speedup vs baseline: 1.7991x; 1.7991x over previous
"""Trainium2 Bass kernel for nn_DMGAGRUcell (GRU cell with graph-conv gates).

Math (per batch b):
  x    = [inputs | hx]                      (N, 66)
  x1   = S @ x, x2 = adp[b] @ x             (diffusion + adaptive hop)
  ru   = sigmoid([x|x1|x2]_interleaved @ W_ru);  r, u = split(ru)
  c    = tanh([x|x1|x2']_interleaved @ W_c)  with x' = [inputs | r*hx]
  out  = u*hx + (1-u)*c

Sharding: 2 batches per core x 8 cores (data parallel over B=16).

Device strategy: the four big N x N streams per batch (S@hx, adp@hx, S@rh,
adp@rh) run as fp8e4m3 DoubleRow matmuls (two 128-node k-chunks per
instruction, 0.5 cycles/row -> 4x bf16 stream throughput). The tiny
inp-feature parts (S@inp, adp@inp, 2 cols) are computed exactly on the host
and shipped as 2-row tiles. fp8 scale factors (S*16, adp*2048) are folded
into the gate weights host-side. Feature rows are reordered to [hx | inp]
so every drain lands at partition base 0. S/adp arrive as 512-column
stripes (all 16 k-chunks per stripe) so gconv1 and the ru gate pipeline
behind the DMA; gates run bf16.
"""

import numpy as np
import ml_dtypes

BF16 = ml_dtypes.bfloat16
FP8 = ml_dtypes.float8_e4m3fn

N = 2048
B = 16
D_IN = 2
UNITS = 64
F = 66
B_LOC = 2          # batches per core
N_CORES = 8
KC = 16            # k chunks of 128 nodes
JP = 8             # k-chunk pairs (DoubleRow)
NG = 8             # 256-col output groups per stream
NS = 4             # 512-wide gate slabs

_CACHE = {}


def _build():
    if "nc" in _CACHE:
        return _CACHE["nc"]

    from contextlib import ExitStack
    import concourse.mybir as mybir
    import concourse.tile as tile
    from concourse import bacc

    f32 = mybir.dt.float32
    bf = mybir.dt.bfloat16
    f8 = mybir.dt.float8e4
    AF = mybir.ActivationFunctionType
    DR = mybir.MatmulPerfMode.DoubleRow

    nc = bacc.Bacc("TRN2", target_bir_lowering=False, debug=False,
                   num_devices=N_CORES)

    adp8_d = nc.dram_tensor("adp8", [B_LOC, 128, KC, N], f8, kind="ExternalInput")
    s8_d = nc.dram_tensor("s8", [1, 128, KC, N], f8, kind="ExternalInput")
    xh8_d = nc.dram_tensor("xh8", [B_LOC, 128, KC, UNITS], f8, kind="ExternalInput")
    x0T_d = nc.dram_tensor("x0T", [B_LOC, F, N], bf, kind="ExternalInput")
    si_d = nc.dram_tensor("si", [B_LOC, D_IN, N], bf, kind="ExternalInput")
    ai_d = nc.dram_tensor("ai", [B_LOC, D_IN, N], bf, kind="ExternalInput")
    wru_d = nc.dram_tensor("wru", [F, 384], bf, kind="ExternalInput")
    wc_d = nc.dram_tensor("wc", [F, 192], bf, kind="ExternalInput")
    id8_d = nc.dram_tensor("id8", [UNITS, UNITS], bf, kind="ExternalInput")
    out_d = nc.dram_tensor("outT", [B_LOC, UNITS, N], bf, kind="ExternalOutput")

    with tile.TileContext(nc) as tc, ExitStack() as ctx:
        cpool = ctx.enter_context(tc.tile_pool(name="cpool", bufs=1))
        rpool = ctx.enter_context(tc.tile_pool(name="rpool", bufs=2))
        tpool = ctx.enter_context(tc.tile_pool(name="tpool", bufs=2))
        pp_s = ctx.enter_context(tc.tile_pool(name="pp_s", bufs=2, space="PSUM"))
        pp_ru = ctx.enter_context(tc.tile_pool(name="pp_ru", bufs=2, space="PSUM"))
        pp_c = ctx.enter_context(tc.tile_pool(name="pp_c", bufs=2, space="PSUM"))
        pp_t = ctx.enter_context(tc.tile_pool(name="pp_t", bufs=2, space="PSUM"))

        # ---- SBUF tiles ----
        s8_t = cpool.tile([128, KC, N], f8, tag="s8", name="s8_t")
        adp_t = [cpool.tile([128, KC, N], f8, tag=f"adp{b}", name=f"adp_t{b}")
                 for b in range(B_LOC)]
        xh8 = [cpool.tile([128, KC, UNITS], f8, tag=f"xh8_{b}", name=f"xh8_{b}")
               for b in range(B_LOC)]
        x0T = [cpool.tile([F, N], bf, tag=f"x0T_{b}", name=f"x0T_{b}")
               for b in range(B_LOC)]
        x1T = [cpool.tile([F, N], bf, tag=f"x1T_{b}", name=f"x1T_{b}")
               for b in range(B_LOC)]
        x2T = [cpool.tile([F, N], bf, tag=f"x2T_{b}", name=f"x2T_{b}")
               for b in range(B_LOC)]
        x0p = [cpool.tile([F, N], bf, tag=f"x0p_{b}", name=f"x0p_{b}")
               for b in range(B_LOC)]
        rh8nm = [cpool.tile([128, KC, UNITS], f8, tag=f"rh8nm_{b}",
                            name=f"rh8nm_{b}") for b in range(B_LOC)]
        uact = [cpool.tile([UNITS, N], bf, tag=f"uact_{b}", name=f"uact_{b}")
                for b in range(B_LOC)]
        cT = [cpool.tile([UNITS, N], f32, tag=f"cT_{b}", name=f"cT_{b}")
              for b in range(B_LOC)]
        outT = [cpool.tile([UNITS, N], bf, tag=f"outT_{b}", name=f"outT_{b}")
                for b in range(B_LOC)]
        wru = cpool.tile([F, 384], bf, tag="wru", name="wru")
        wc = cpool.tile([F, 192], bf, tag="wc", name="wc")
        id8 = cpool.tile([UNITS, UNITS], bf, tag="id8", name="id8")

        # ---- DMA order = arrival order (SP queue is in-order). ----
        # lhsT first, then S / adp[0] in 512-col stripes (all 16 k-chunks per
        # stripe -> full contraction available per stripe), with the small
        # gate inputs slotted between stripes, then adp[1].
        nc.sync.dma_start(xh8[0][:], xh8_d[0])
        nc.sync.dma_start(xh8[1][:], xh8_d[1])

        def smalls(b):
            nc.sync.dma_start(x0T[b][:], x0T_d[b])
            nc.sync.dma_start(x1T[b][UNITS:F, :], si_d[b])
            nc.sync.dma_start(x2T[b][UNITS:F, :], ai_d[b])
            nc.sync.dma_start(x0p[b][UNITS:F, :], x0T_d[b][UNITS:F, :])

        for s in range(NS):
            sl = slice(s * 512, (s + 1) * 512)
            nc.sync.dma_start(s8_t[:, :, sl], s8_d[0][:, :, sl])
            nc.sync.dma_start(adp_t[0][:, :, sl], adp8_d[0][:, :, sl])
            if s == 0:
                smalls(0)
                nc.sync.dma_start(wru[:], wru_d[:])
                nc.sync.dma_start(wc[:], wc_d[:])
                nc.sync.dma_start(id8[:], id8_d[:])
            if s == 3:
                smalls(1)
        for s in range(NS):
            sl = slice(s * 512, (s + 1) * 512)
            nc.sync.dma_start(adp_t[1][:, :, sl], adp8_d[1][:, :, sl])

        # warm the ACT function table off the critical path
        dum = cpool.tile([1, 2], f32, tag="dum", name="dum")
        nc.scalar.activation(dum[0:1, 0:1], id8[0:1, 0:1], AF.Sigmoid)
        nc.scalar.activation(dum[0:1, 1:2], id8[0:1, 0:1], AF.Tanh)

        def dr_group(lhsT, rhs, dst, g, pfx):
            # one 256-col output group of a full N x N fp8 DoubleRow stream
            c0 = g * 256
            ps = pp_s.tile([UNITS, 512], f32, tag="ps_s", name=f"ps_{pfx}_{g}")
            for j in range(JP):
                nc.tensor.matmul(
                    ps[:, 0:256], lhsT[:, 2 * j:2 * j + 2, :],
                    rhs[:, 2 * j:2 * j + 2, c0:c0 + 256],
                    start=(j == 0), stop=(j == JP - 1), perf_mode=DR)
            nc.vector.tensor_copy(dst[0:UNITS, c0:c0 + 256], ps[:, 0:256])

        def ru_slab(b, s):
            sl = slice(s * 512, (s + 1) * 512)
            ps = pp_ru.tile([128, 512], f32, tag="ps_ru", name=f"ps_ru{b}_{s}")
            nc.tensor.matmul(ps[:], wru[:, 0:128], x0T[b][:, sl],
                             start=True, stop=False)
            nc.tensor.matmul(ps[:], wru[:, 128:256], x1T[b][:, sl],
                             start=False, stop=False)
            nc.tensor.matmul(ps[:], wru[:, 256:384], x2T[b][:, sl],
                             start=False, stop=True)
            rt = rpool.tile([UNITS, 512], f32, tag="rt", name=f"rt{b}_{s}")
            nc.scalar.activation(rt[:], ps[0:UNITS, :], AF.Sigmoid)
            # rh = r*hx -> bf16 gate copy (rows 0-63 of x0p) + fp8 stream copy
            nc.vector.tensor_mul(x0p[b][0:UNITS, sl], rt[:], x0T[b][0:UNITS, sl])
            nc.scalar.activation(uact[b][:, sl], ps[UNITS:128, :], AF.Sigmoid)

        def transposes(b):
            for blk in range(KC):
                pt = pp_t.tile([128, 1024], bf, tag="ps_t", name=f"pt{b}_{blk}")
                nc.tensor.transpose(
                    pt[:, 0:UNITS],
                    x0p[b][0:UNITS, blk * 128:(blk + 1) * 128], id8[:])
                nc.vector.tensor_copy(rh8nm[b][:, blk, :], pt[:, 0:UNITS])

        def c_slab(b, s):
            sl = slice(s * 512, (s + 1) * 512)
            ps = pp_c.tile([UNITS, 512], f32, tag="ps_c", name=f"ps_c{b}_{s}")
            nc.tensor.matmul(ps[:], wc[:, 0:64], x0p[b][:, sl],
                             start=True, stop=False)
            nc.tensor.matmul(ps[:], wc[:, 64:128], x1T[b][:, sl],
                             start=False, stop=False)
            nc.tensor.matmul(ps[:], wc[:, 128:192], x2T[b][:, sl],
                             start=False, stop=True)
            nc.scalar.activation(cT[b][:, sl], ps[:], AF.Tanh)
            # out = c + u*(hx - c) on GpSimd (keeps DVE free for drains)
            tm = tpool.tile([UNITS, 512], f32, tag="tm", name=f"tm{b}_{s}")
            nc.gpsimd.tensor_sub(tm[:], x0T[b][0:UNITS, sl], cT[b][:, sl])
            nc.gpsimd.tensor_mul(tm[:], uact[b][:, sl], tm[:])
            nc.gpsimd.tensor_add(outT[b][:, sl], tm[:], cT[b][:, sl])

        # ---- gconv1, pipelined behind the S/adp[0] stripe DMAs ----
        for s in range(NS):
            for h in range(2):
                g = 2 * s + h
                dr_group(xh8[0], s8_t, x1T[0], g, "s1b0")
                dr_group(xh8[1], s8_t, x1T[1], g, "s1b1")
                dr_group(xh8[0], adp_t[0], x2T[0], g, "a1b0")
            ru_slab(0, s)

        # ---- batch 0 gconv2 + c, interleaved with batch 1 gconv1-adp ----
        transposes(0)
        for g in range(NG):
            dr_group(rh8nm[0], s8_t, x1T[0], g, "s2b0")
        for s in range(NS):
            dr_group(rh8nm[0], adp_t[0], x2T[0], 2 * s, "a2b0")
            dr_group(rh8nm[0], adp_t[0], x2T[0], 2 * s + 1, "a2b0")
            c_slab(0, s)
        for s in range(NS):
            dr_group(xh8[1], adp_t[1], x2T[1], 2 * s, "a1b1")
            dr_group(xh8[1], adp_t[1], x2T[1], 2 * s + 1, "a1b1")
            ru_slab(1, s)

        # ---- batch 1 gconv2 + c ----
        transposes(1)
        for g in range(NG):
            dr_group(rh8nm[1], s8_t, x1T[1], g, "s2b1")
        for s in range(NS):
            dr_group(rh8nm[1], adp_t[1], x2T[1], 2 * s, "a2b1")
            dr_group(rh8nm[1], adp_t[1], x2T[1], 2 * s + 1, "a2b1")
            c_slab(1, s)

        nc.sync.dma_start(out_d[0], outT[0][:])
        nc.sync.dma_start(out_d[1], outT[1][:])

    nc.compile()
    _CACHE["nc"] = nc
    return nc


def _prep_host(inputs, hx, adp, support_rows, support_cols, support_vals,
               W_ru, W_c):
    inp = np.ascontiguousarray(inputs.reshape(B, N, D_IN), np.float32)
    hxm = np.ascontiguousarray(hx.reshape(B, N, UNITS), np.float32)

    S = np.zeros((N, N), np.float32)
    np.add.at(S, (support_rows, support_cols), support_vals)

    # fp8 stream operands; scales (x16 / x2048) are folded into the weights
    s8 = (S.T * 16.0).astype(FP8).reshape(KC, 128, N).transpose(1, 0, 2)
    s8 = np.ascontiguousarray(s8)[None]
    adp8 = np.empty((B, 128, KC, N), FP8)
    for b in range(B):
        adp8[b] = (adp[b].T * 2048.0).astype(FP8).reshape(
            KC, 128, N).transpose(1, 0, 2)
    xh8 = np.ascontiguousarray(
        hxm.reshape(B, KC, 128, UNITS).transpose(0, 2, 1, 3)).astype(FP8)

    # feature-major gate inputs, rows reordered to [hx | inp]
    x0T = np.concatenate(
        [hxm.transpose(0, 2, 1), inp.transpose(0, 2, 1)], axis=1).astype(BF16)
    # exact inp-feature parts of the streams (reused by both gconvs)
    si = (16.0 * np.einsum('nm,bmf->bfn', S, inp)).astype(BF16)
    ai = (2048.0 * np.einsum('bnm,bmf->bfn', adp, inp)).astype(BF16)

    def reord(w):  # rows [inp(2) | hx(64)] -> [hx | inp]
        return np.concatenate([w[D_IN:F], w[0:D_IN]], axis=0)

    wru = np.concatenate(
        [reord(W_ru[0::3]), reord(W_ru[1::3]) / 16.0,
         reord(W_ru[2::3]) / 2048.0], axis=1).astype(BF16)
    wc = np.concatenate(
        [reord(W_c[0::3]), reord(W_c[1::3]) / 16.0,
         reord(W_c[2::3]) / 2048.0], axis=1).astype(BF16)
    id8 = np.eye(UNITS, dtype=BF16)

    shared = {"s8": s8, "wru": np.ascontiguousarray(wru),
              "wc": np.ascontiguousarray(wc), "id8": id8}
    in_maps = []
    for c in range(N_CORES):
        lo, hi = c * B_LOC, (c + 1) * B_LOC
        in_maps.append({
            "adp8": np.ascontiguousarray(adp8[lo:hi]),
            "xh8": np.ascontiguousarray(xh8[lo:hi]),
            "x0T": np.ascontiguousarray(x0T[lo:hi]),
            "si": np.ascontiguousarray(si[lo:hi]),
            "ai": np.ascontiguousarray(ai[lo:hi]),
            **shared,
        })
    return in_maps


def kernel(inputs, hx, adp, support_rows, support_cols, support_vals,
           W_ru, W_c, time_axis=None):
    from concourse.bass_utils import run_bass_kernel_spmd

    inputs = np.asarray(inputs, dtype=np.float32)
    hx = np.asarray(hx, dtype=np.float32)
    adp = np.asarray(adp, dtype=np.float32)
    support_rows = np.asarray(support_rows)
    support_cols = np.asarray(support_cols)
    support_vals = np.asarray(support_vals, dtype=np.float32)
    W_ru = np.asarray(W_ru, dtype=np.float32)
    W_c = np.asarray(W_c, dtype=np.float32)

    nc = _build()
    in_maps = _prep_host(inputs, hx, adp, support_rows, support_cols,
                         support_vals, W_ru, W_c)

    res = run_bass_kernel_spmd(nc, in_maps, core_ids=list(range(N_CORES)),
                               trace=False)
    _CACHE["last_result"] = res

    out = np.empty((B, N * UNITS), np.float32)
    for c in range(N_CORES):
        outT = res.results[c]["outT"]  # (B_LOC, 64, N) bf16
        for i in range(B_LOC):
            out[c * B_LOC + i] = np.ascontiguousarray(
                outT[i].astype(np.float32).T).reshape(N * UNITS)
    return out


# revision 5
# speedup vs baseline: 1.8758x; 1.0426x over previous
"""Trainium2 Bass kernel for nn_DMGAGRUcell (GRU cell with graph-conv gates).

Math (per batch b):
  x    = [inputs | hx]                      (N, 66)
  x1   = S @ x, x2 = adp[b] @ x             (diffusion + adaptive hop)
  ru   = sigmoid([x|x1|x2]_interleaved @ W_ru);  r, u = split(ru)
  c    = tanh([x|x1|x2']_interleaved @ W_c)  with x' = [inputs | r*hx]
  out  = u*hx + (1-u)*c

Sharding: 2 batches per core x 8 cores (data parallel over B=16).

Device strategy: the four big N x N streams per batch (S@hx, adp@hx, S@rh,
adp@rh) run as fp8e4m3 DoubleRow matmuls (two 128-node k-chunks per
instruction, 0.5 cycles/row -> 4x bf16 stream throughput). The tiny
inp-feature parts (S@inp, adp@inp, 2 cols) are computed exactly on the host
and shipped as 2-row tiles. fp8 scale factors (S*16, adp*2048) are folded
into the gate weights host-side. Feature rows are reordered to [hx | inp]
so every drain lands at partition base 0. S/adp arrive as 512-column
stripes (all 16 k-chunks per stripe) so gconv1 and the ru gate pipeline
behind the DMA; gates run bf16.
"""

import numpy as np
import ml_dtypes

BF16 = ml_dtypes.bfloat16
FP8 = ml_dtypes.float8_e4m3fn

N = 2048
B = 16
D_IN = 2
UNITS = 64
F = 66
B_LOC = 2          # batches per core
N_CORES = 8
KC = 16            # k chunks of 128 nodes
JP = 8             # k-chunk pairs (DoubleRow)
NG = 8             # 256-col output groups per stream
NS = 4             # 512-wide gate slabs

_CACHE = {}


def _build():
    if "nc" in _CACHE:
        return _CACHE["nc"]

    from contextlib import ExitStack
    import concourse.mybir as mybir
    import concourse.tile as tile
    from concourse import bacc

    f32 = mybir.dt.float32
    bf = mybir.dt.bfloat16
    f8 = mybir.dt.float8e4
    AF = mybir.ActivationFunctionType
    DR = mybir.MatmulPerfMode.DoubleRow

    nc = bacc.Bacc("TRN2", target_bir_lowering=False, debug=False,
                   num_devices=N_CORES)

    adp8_d = nc.dram_tensor("adp8", [B_LOC, 128, KC, N], f8, kind="ExternalInput")
    s8_d = nc.dram_tensor("s8", [1, 128, KC, N], f8, kind="ExternalInput")
    xh8_d = nc.dram_tensor("xh8", [B_LOC, 128, KC, UNITS], f8, kind="ExternalInput")
    x0T_d = nc.dram_tensor("x0T", [B_LOC, F, N], bf, kind="ExternalInput")
    si_d = nc.dram_tensor("si", [B_LOC, D_IN, N], bf, kind="ExternalInput")
    ai_d = nc.dram_tensor("ai", [B_LOC, D_IN, N], bf, kind="ExternalInput")
    wru_d = nc.dram_tensor("wru", [F, 384], bf, kind="ExternalInput")
    wc_d = nc.dram_tensor("wc", [F, 192], bf, kind="ExternalInput")
    id8_d = nc.dram_tensor("id8", [UNITS, UNITS], bf, kind="ExternalInput")
    out_d = nc.dram_tensor("outT", [B_LOC, UNITS, N], bf, kind="ExternalOutput")

    with tile.TileContext(nc) as tc, ExitStack() as ctx:
        cpool = ctx.enter_context(tc.tile_pool(name="cpool", bufs=1))
        rpool = ctx.enter_context(tc.tile_pool(name="rpool", bufs=2))
        tpool = ctx.enter_context(tc.tile_pool(name="tpool", bufs=2))
        pp_s = ctx.enter_context(tc.tile_pool(name="pp_s", bufs=2, space="PSUM"))
        pp_ru = ctx.enter_context(tc.tile_pool(name="pp_ru", bufs=2, space="PSUM"))
        pp_c = ctx.enter_context(tc.tile_pool(name="pp_c", bufs=2, space="PSUM"))
        pp_t = ctx.enter_context(tc.tile_pool(name="pp_t", bufs=2, space="PSUM"))

        # ---- SBUF tiles ----
        s8_t = cpool.tile([128, KC, N], f8, tag="s8", name="s8_t")
        adp_t = [cpool.tile([128, KC, N], f8, tag=f"adp{b}", name=f"adp_t{b}")
                 for b in range(B_LOC)]
        xh8 = [cpool.tile([128, KC, UNITS], f8, tag=f"xh8_{b}", name=f"xh8_{b}")
               for b in range(B_LOC)]
        x0T = [cpool.tile([F, N], bf, tag=f"x0T_{b}", name=f"x0T_{b}")
               for b in range(B_LOC)]
        x1T = [cpool.tile([F, N], bf, tag=f"x1T_{b}", name=f"x1T_{b}")
               for b in range(B_LOC)]
        x2T = [cpool.tile([F, N], bf, tag=f"x2T_{b}", name=f"x2T_{b}")
               for b in range(B_LOC)]
        x0p = [cpool.tile([F, N], bf, tag=f"x0p_{b}", name=f"x0p_{b}")
               for b in range(B_LOC)]
        rh8nm = [cpool.tile([128, KC, UNITS], f8, tag=f"rh8nm_{b}",
                            name=f"rh8nm_{b}") for b in range(B_LOC)]
        uact = [cpool.tile([UNITS, N], bf, tag=f"uact_{b}", name=f"uact_{b}")
                for b in range(B_LOC)]
        cT = [cpool.tile([UNITS, N], bf, tag=f"cT_{b}", name=f"cT_{b}")
              for b in range(B_LOC)]
        outT = [cpool.tile([UNITS, N], bf, tag=f"outT_{b}", name=f"outT_{b}")
                for b in range(B_LOC)]
        wru = cpool.tile([F, 384], bf, tag="wru", name="wru")
        wc = cpool.tile([F, 192], bf, tag="wc", name="wc")
        id8 = cpool.tile([UNITS, UNITS], bf, tag="id8", name="id8")

        # ---- DMA order = arrival order (SP queue is in-order). ----
        # lhsT first, then S / adp[0] in 512-col stripes (all 16 k-chunks per
        # stripe -> full contraction available per stripe), with the small
        # gate inputs slotted between stripes, then adp[1].
        nc.sync.dma_start(xh8[0][:], xh8_d[0])
        nc.sync.dma_start(xh8[1][:], xh8_d[1])

        def smalls(b):
            nc.sync.dma_start(x0T[b][:], x0T_d[b])
            nc.sync.dma_start(x1T[b][UNITS:F, :], si_d[b])
            nc.sync.dma_start(x2T[b][UNITS:F, :], ai_d[b])
            nc.sync.dma_start(x0p[b][UNITS:F, :], x0T_d[b][UNITS:F, :])

        for s in range(NS):
            sl = slice(s * 512, (s + 1) * 512)
            nc.sync.dma_start(s8_t[:, :, sl], s8_d[0][:, :, sl])
            nc.sync.dma_start(adp_t[0][:, :, sl], adp8_d[0][:, :, sl])
        smalls(0)
        nc.sync.dma_start(wru[:], wru_d[:])
        nc.sync.dma_start(wc[:], wc_d[:])
        nc.sync.dma_start(id8[:], id8_d[:])
        for s in range(NS):
            sl = slice(s * 512, (s + 1) * 512)
            nc.sync.dma_start(adp_t[1][:, :, sl], adp8_d[1][:, :, sl])
            if s == 0:
                smalls(1)

        # warm the ACT function table off the critical path
        dum = cpool.tile([1, 2], f32, tag="dum", name="dum")
        nc.scalar.activation(dum[0:1, 0:1], id8[0:1, 0:1], AF.Sigmoid)
        nc.scalar.activation(dum[0:1, 1:2], id8[0:1, 0:1], AF.Tanh)

        def dr_pair(lhsT, rhs, dst, s, pfx, eng):
            # one 512-col slab of a full N x N fp8 DoubleRow stream: two
            # 256-col halves share one psum bank (one accumulation zone),
            # drained by a single 512-wide copy on DVE or ACT.
            ps = pp_s.tile([UNITS, 512], f32, tag="ps_s", name=f"ps_{pfx}_{s}")
            for h in range(2):
                c0 = s * 512 + h * 256
                for j in range(JP):
                    nc.tensor.matmul(
                        ps[:, h * 256:h * 256 + 256],
                        lhsT[:, 2 * j:2 * j + 2, :],
                        rhs[:, 2 * j:2 * j + 2, c0:c0 + 256],
                        start=(h == 0 and j == 0),
                        stop=(h == 1 and j == JP - 1), perf_mode=DR)
            dsl = slice(s * 512, (s + 1) * 512)
            if eng == "act":
                nc.scalar.activation(dst[0:UNITS, dsl], ps[:], AF.Copy)
            else:
                nc.vector.tensor_copy(dst[0:UNITS, dsl], ps[:])

        def ru_slab(b, s):
            sl = slice(s * 512, (s + 1) * 512)
            ps = pp_ru.tile([128, 512], f32, tag="ps_ru", name=f"ps_ru{b}_{s}")
            nc.tensor.matmul(ps[:], wru[:, 0:128], x0T[b][:, sl],
                             start=True, stop=False)
            nc.tensor.matmul(ps[:], wru[:, 128:256], x1T[b][:, sl],
                             start=False, stop=False)
            nc.tensor.matmul(ps[:], wru[:, 256:384], x2T[b][:, sl],
                             start=False, stop=True)
            rt = rpool.tile([UNITS, 512], f32, tag="rt", name=f"rt{b}_{s}")
            nc.scalar.activation(rt[:], ps[0:UNITS, :], AF.Sigmoid)
            # rh = r*hx -> bf16 gate copy (rows 0-63 of x0p) + fp8 stream copy
            nc.vector.tensor_mul(x0p[b][0:UNITS, sl], rt[:], x0T[b][0:UNITS, sl])
            nc.scalar.activation(uact[b][:, sl], ps[UNITS:128, :], AF.Sigmoid)

        def transposes(b):
            for blk in range(KC):
                pt = pp_t.tile([128, 1024], bf, tag="ps_t", name=f"pt{b}_{blk}")
                nc.tensor.transpose(
                    pt[:, 0:UNITS],
                    x0p[b][0:UNITS, blk * 128:(blk + 1) * 128], id8[:])
                if blk % 2 == 0:
                    nc.vector.tensor_copy(rh8nm[b][:, blk, :], pt[:, 0:UNITS])
                else:
                    nc.scalar.activation(rh8nm[b][:, blk, :], pt[:, 0:UNITS],
                                         AF.Copy)

        def c_slab(b, s):
            sl = slice(s * 512, (s + 1) * 512)
            ps = pp_c.tile([UNITS, 512], f32, tag="ps_c", name=f"ps_c{b}_{s}")
            nc.tensor.matmul(ps[:], wc[:, 0:64], x0p[b][:, sl],
                             start=True, stop=False)
            nc.tensor.matmul(ps[:], wc[:, 64:128], x1T[b][:, sl],
                             start=False, stop=False)
            nc.tensor.matmul(ps[:], wc[:, 128:192], x2T[b][:, sl],
                             start=False, stop=True)
            nc.scalar.activation(cT[b][:, sl], ps[:], AF.Tanh)
            # out = c + u*(hx - c), bf16; odd slabs DVE, even slabs GpSimd
            eng = nc.vector if s % 2 == 1 else nc.gpsimd
            tm = tpool.tile([UNITS, 512], bf, tag="tm", name=f"tm{b}_{s}")
            eng.tensor_sub(tm[:], x0T[b][0:UNITS, sl], cT[b][:, sl])
            eng.tensor_mul(tm[:], uact[b][:, sl], tm[:])
            eng.tensor_add(outT[b][:, sl], tm[:], cT[b][:, sl])

        # ---- gconv1, pipelined behind the S/adp[0] stripe DMAs ----
        for s in range(NS):
            dr_pair(xh8[0], s8_t, x1T[0], s, "s1b0", "dve")
            dr_pair(xh8[1], s8_t, x1T[1], s, "s1b1", "act")
            dr_pair(xh8[0], adp_t[0], x2T[0], s, "a1b0", "dve")
            ru_slab(0, s)

        # ---- batch 0 gconv2 + c, interleaved with batch 1 gconv1-adp ----
        transposes(0)
        for s in range(NS):
            dr_pair(rh8nm[0], s8_t, x1T[0], s, "s2b0",
                    "act" if s % 2 else "dve")
        for s in range(NS):
            dr_pair(rh8nm[0], adp_t[0], x2T[0], s, "a2b0",
                    "dve" if s % 2 else "act")
            c_slab(0, s)
        for s in range(NS):
            dr_pair(xh8[1], adp_t[1], x2T[1], s, "a1b1",
                    "act" if s % 2 else "dve")
            ru_slab(1, s)

        # ---- batch 1 gconv2 + c ----
        transposes(1)
        for s in range(NS):
            dr_pair(rh8nm[1], s8_t, x1T[1], s, "s2b1",
                    "act" if s % 2 else "dve")
        for s in range(NS):
            dr_pair(rh8nm[1], adp_t[1], x2T[1], s, "a2b1",
                    "dve" if s % 2 else "act")
            c_slab(1, s)

        nc.sync.dma_start(out_d[0], outT[0][:])
        nc.sync.dma_start(out_d[1], outT[1][:])

    nc.compile()
    _CACHE["nc"] = nc
    return nc


def _prep_host(inputs, hx, adp, support_rows, support_cols, support_vals,
               W_ru, W_c):
    inp = np.ascontiguousarray(inputs.reshape(B, N, D_IN), np.float32)
    hxm = np.ascontiguousarray(hx.reshape(B, N, UNITS), np.float32)

    S = np.zeros((N, N), np.float32)
    np.add.at(S, (support_rows, support_cols), support_vals)

    # fp8 stream operands; scales (x16 / x2048) are folded into the weights
    s8 = (S.T * 16.0).astype(FP8).reshape(KC, 128, N).transpose(1, 0, 2)
    s8 = np.ascontiguousarray(s8)[None]
    adp8 = np.empty((B, 128, KC, N), FP8)
    for b in range(B):
        adp8[b] = (adp[b].T * 2048.0).astype(FP8).reshape(
            KC, 128, N).transpose(1, 0, 2)
    xh8 = np.ascontiguousarray(
        hxm.reshape(B, KC, 128, UNITS).transpose(0, 2, 1, 3)).astype(FP8)

    # feature-major gate inputs, rows reordered to [hx | inp]
    x0T = np.concatenate(
        [hxm.transpose(0, 2, 1), inp.transpose(0, 2, 1)], axis=1).astype(BF16)
    # exact inp-feature parts of the streams (reused by both gconvs)
    si = (16.0 * np.einsum('nm,bmf->bfn', S, inp)).astype(BF16)
    ai = (2048.0 * np.einsum('bnm,bmf->bfn', adp, inp)).astype(BF16)

    def reord(w):  # rows [inp(2) | hx(64)] -> [hx | inp]
        return np.concatenate([w[D_IN:F], w[0:D_IN]], axis=0)

    wru = np.concatenate(
        [reord(W_ru[0::3]), reord(W_ru[1::3]) / 16.0,
         reord(W_ru[2::3]) / 2048.0], axis=1).astype(BF16)
    wc = np.concatenate(
        [reord(W_c[0::3]), reord(W_c[1::3]) / 16.0,
         reord(W_c[2::3]) / 2048.0], axis=1).astype(BF16)
    id8 = np.eye(UNITS, dtype=BF16)

    shared = {"s8": s8, "wru": np.ascontiguousarray(wru),
              "wc": np.ascontiguousarray(wc), "id8": id8}
    in_maps = []
    for c in range(N_CORES):
        lo, hi = c * B_LOC, (c + 1) * B_LOC
        in_maps.append({
            "adp8": np.ascontiguousarray(adp8[lo:hi]),
            "xh8": np.ascontiguousarray(xh8[lo:hi]),
            "x0T": np.ascontiguousarray(x0T[lo:hi]),
            "si": np.ascontiguousarray(si[lo:hi]),
            "ai": np.ascontiguousarray(ai[lo:hi]),
            **shared,
        })
    return in_maps


def kernel(inputs, hx, adp, support_rows, support_cols, support_vals,
           W_ru, W_c, time_axis=None):
    from concourse.bass_utils import run_bass_kernel_spmd

    inputs = np.asarray(inputs, dtype=np.float32)
    hx = np.asarray(hx, dtype=np.float32)
    adp = np.asarray(adp, dtype=np.float32)
    support_rows = np.asarray(support_rows)
    support_cols = np.asarray(support_cols)
    support_vals = np.asarray(support_vals, dtype=np.float32)
    W_ru = np.asarray(W_ru, dtype=np.float32)
    W_c = np.asarray(W_c, dtype=np.float32)

    nc = _build()
    in_maps = _prep_host(inputs, hx, adp, support_rows, support_cols,
                         support_vals, W_ru, W_c)

    res = run_bass_kernel_spmd(nc, in_maps, core_ids=list(range(N_CORES)),
                               trace=False)
    _CACHE["last_result"] = res

    out = np.empty((B, N * UNITS), np.float32)
    for c in range(N_CORES):
        outT = res.results[c]["outT"]  # (B_LOC, 64, N) bf16
        for i in range(B_LOC):
            out[c * B_LOC + i] = np.ascontiguousarray(
                outT[i].astype(np.float32).T).reshape(N * UNITS)
    return out
